# revision 1
# baseline (speedup 1.0000x reference)
"""Trainium2 Bass kernel for CNN + node-attention + per-cell embedding gather.

Reference computation (B=32, N=32, E=128, CIN=64, COUT=128, H=W=128):
  1. conv_out = Conv2d(state, conv_w, 3x3, pad 1) + conv_b          [B,COUT,H,W]
  2. node attention over N nodes -> out_node                        [B,N,COUT]
  3. out = conv_out + out_node[b, char_to_node[game_board]] + final_b (gather)

Sharding: data-parallel over batch, 4 batches per core on 8 cores.

Device strategy per 32-row block of one batch image (bf16 streams, fp32 PSUM):
  - state is held in SBUF in a 130-wide zero-padded layout; tile A holds the
    padded rows on partitions 0..63 and the same data shifted one image row
    on partitions 64..127 (one SBUF->SBUF DMA).  The 9 conv taps then reduce
    to 6 matmuls per PSUM tile, all accumulating into one PSUM bank:
      3x K=128 [W(0,dx); W(1,dx)] @ A[:, j+dx]        (6 taps)
      2x K=64  W(2,dx)            @ A[0:64, j+260+dx] (dx=0,1)
      1x K=97  [W(2,2); out_node; bias] @ C[0:97, j+262]
  - tile C = [copy of padded state; one-hot rows; all-ones row].  The K=97
    matmul fuses the last conv tap, the per-cell embedding gather (one-hot
    matmul against host-precomputed one-hot rows), and the (conv_b+final_b)
    bias (ones row) into the same accumulation.
  - ACT/DVE alternate evacuating PSUM to an unpadded f32 staging tile
    (dropping pad columns); one contiguous 2MB DMA stores each block.
"""

import os

import numpy as np
import ml_dtypes

import concourse.bass as bass
from concourse import bacc
import concourse.mybir as mybir
from concourse.bass_utils import run_bass_kernel_spmd
from concourse.masks import make_identity
from concourse.tile import TileContext

# Problem constants (hardcoded; kernel.py must be self-contained).
B, N, E, CIN, COUT, H, W, KS = 32, 32, 128, 64, 128, 128, 128, 3
NCORES = 8
BPC = B // NCORES           # batches per core
R = 64                      # output rows per block
NBLK = H // R               # blocks per batch
WP = W + 2                  # padded width (130)
JBLK = R * WP               # padded output columns per block (4160)
NT = 3 * WP                 # psum tile width (390): 3 full padded rows
SROWS = R + 2               # state rows held per block (34)
SLEN = 1 + SROWS * WP + 1   # state tile free size (1 lead + 34*130 + 1 tail)
NSLOT = None                # set from NSLOT_A below

F32 = mybir.dt.float32
BF16 = mybir.dt.bfloat16
STORE_BF16 = os.environ.get("K_STORE_BF16", "1") == "1"
NSLOT_A = int(os.environ.get("K_NSLOT", "2"))
PP_BUFS = int(os.environ.get("K_PP", "5"))
ATTN_PS = int(os.environ.get("K_ATTNPS", "3"))
EVAC_MOD = int(os.environ.get("K_EVAC", "0"))

NSLOT = None
_CACHE = {}
LAST_RESULTS = None         # test.py reads timing info from here


def _psum_tiles():
    """(col_offset, width) chunks of one block's padded output columns."""
    out, j = [], 0
    while j < JBLK:
        w = min(NT, JBLK - j)
        out.append((j, w))
        j += w
    return out


def _build_attention(nc, sb, ps, consts, b, combo):
    """Emit node-attention for one batch; writes combo[64:96, :] = out_node
    (lhsT layout [N, COUT], bf16) and combo[96, :] = bias row."""
    node_t = consts["node_t"]          # [E, N]   x_lo^T
    goal_bc = consts["goal_bc"][b]     # [E, N]   x_hi^T (goal broadcast)
    wq_lo, wq_hi = consts["wq_lo"], consts["wq_hi"]
    wk_lo, wk_hi = consts["wk_lo"], consts["wk_hi"]
    wv_lo, wv_hi = consts["wv_lo"], consts["wv_hi"]
    fw_t = consts["fw_t"]              # [E, COUT]
    ident = consts["ident"]            # [128, 128] f32
    bias_row = consts["bias_row"]      # [1, COUT] bf16

    # Q^T, K^T: [E, N] = (x @ wQ)^T ; accumulate the two K-halves.
    qt_ps = ps.tile([128, N], F32, tag="aps", name=f"qt_ps{b}")
    nc.tensor.matmul(out=qt_ps[:], lhsT=wq_lo, rhs=node_t[:], start=True, stop=False)
    nc.tensor.matmul(out=qt_ps[:], lhsT=wq_hi, rhs=goal_bc[:], start=False, stop=True)
    qt_sb = sb.tile([128, N], F32, tag="qt_sb", name=f"qt_sb{b}")
    nc.vector.tensor_scalar_mul(qt_sb[:], qt_ps[:], float(1.0 / np.sqrt(float(E))))

    kt_ps = ps.tile([128, N], F32, tag="aps", name=f"kt_ps{b}")
    nc.tensor.matmul(out=kt_ps[:], lhsT=wk_lo, rhs=node_t[:], start=True, stop=False)
    nc.tensor.matmul(out=kt_ps[:], lhsT=wk_hi, rhs=goal_bc[:], start=False, stop=True)
    kt_sb = sb.tile([128, N], F32, tag="kt_sb", name=f"kt_sb{b}")
    nc.vector.tensor_copy(kt_sb[:], kt_ps[:])

    # V: [N, E]
    v_ps = ps.tile([N, E], F32, tag="aps", name=f"v_ps{b}")
    nc.tensor.matmul(out=v_ps[:], lhsT=node_t[:], rhs=wv_lo, start=True, stop=False)
    nc.tensor.matmul(out=v_ps[:], lhsT=goal_bc[:], rhs=wv_hi, start=False, stop=True)
    v_sb = sb.tile([N, E], F32, tag="v_sb", name=f"v_sb{b}")
    nc.vector.tensor_copy(v_sb[:], v_ps[:])

    # scores [N, N] = (Q/sqrt(E)) @ K^T
    sc_ps = ps.tile([N, N], F32, tag="aps", name=f"sc_ps{b}")
    nc.tensor.matmul(out=sc_ps[:], lhsT=qt_sb[:], rhs=kt_sb[:], start=True, stop=True)
    prob = sb.tile([N, N], F32, tag="prob", name=f"prob{b}")
    nc.vector.tensor_copy(prob[:], sc_ps[:])

    # softmax along free axis (in place on prob)
    nmax = sb.tile([N, 1], F32, tag="nmax", name=f"nmax{b}")
    nc.vector.reduce_max(nmax[:], prob[:], axis=mybir.AxisListType.X, negate=True)
    nc.scalar.activation(prob[:], prob[:], mybir.ActivationFunctionType.Exp,
                         bias=nmax[:, 0:1], scale=1.0)
    sum_ = sb.tile([N, 1], F32, tag="sum_", name=f"sum{b}")
    nc.vector.reduce_sum(sum_[:], prob[:], axis=mybir.AxisListType.X)
    rcp = sb.tile([N, 1], F32, tag="rcp", name=f"rcp{b}")
    nc.vector.reciprocal(rcp[:], sum_[:])
    nc.vector.tensor_scalar_mul(prob[:], prob[:], rcp[:, 0:1])

    # P^T [N, N]
    pt_ps = ps.tile([N, N], F32, tag="aps", name=f"pt_ps{b}")
    nc.tensor.transpose(out=pt_ps[:], in_=prob[:], identity=ident[0:N, 0:N])
    pt_sb = sb.tile([N, N], F32, tag="pt_sb", name=f"pt_sb{b}")
    nc.vector.tensor_copy(pt_sb[:], pt_ps[:])

    # atten^T [E, N] = V^T @ P^T
    at_ps = ps.tile([E, N], F32, tag="aps", name=f"at_ps{b}")
    nc.tensor.matmul(out=at_ps[:], lhsT=v_sb[:], rhs=pt_sb[:], start=True, stop=True)
    at_sb = sb.tile([E, N], F32, tag="at_sb", name=f"at_sb{b}")
    nc.vector.tensor_copy(at_sb[:], at_ps[:])

    # out_node [N, COUT] = atten @ final_w^T  -> combo rows 64:96 (bf16 cast)
    on_ps = ps.tile([N, COUT], F32, tag="aps", name=f"on_ps{b}")
    nc.tensor.matmul(out=on_ps[:], lhsT=at_sb[:], rhs=fw_t[:], start=True, stop=True)
    nc.vector.tensor_copy(combo[64:64 + N, :], on_ps[:])
    nc.vector.tensor_copy(combo[96:97, :], bias_row[0:1, :])


def _build_kernel():
    global NSLOT
    NSLOT = NSLOT_A
    nc = bacc.Bacc("TRN2", target_bir_lowering=False, debug=False, num_devices=NCORES)

    state_d = nc.declare_dram_parameter("state", [BPC, CIN, H, W], BF16, isOutput=False)
    onehot_d = nc.declare_dram_parameter("onehot", [BPC, N, H * WP], BF16, isOutput=False)
    wpairs_d = nc.declare_dram_parameter("wpairs", [128, 3 * COUT], BF16, isOutput=False)
    wb_d = nc.declare_dram_parameter("wb", [128, COUT], BF16, isOutput=False)
    w22_d = nc.declare_dram_parameter("w22", [CIN, COUT], BF16, isOutput=False)
    bias_d = nc.declare_dram_parameter("bias_row", [1, COUT], BF16, isOutput=False)
    goal_d = nc.declare_dram_parameter("goal", [BPC, E], F32, isOutput=False)
    node_t_d = nc.declare_dram_parameter("node_t", [E, N], F32, isOutput=False)
    wqkv_d = nc.declare_dram_parameter("wqkv", [2 * E, 3 * E], F32, isOutput=False)
    fw_t_d = nc.declare_dram_parameter("fw_t", [E, COUT], F32, isOutput=False)
    out_d = nc.declare_dram_parameter("out", [BPC, COUT, H * W],
                                     BF16 if STORE_BF16 else F32, isOutput=True)

    with TileContext(nc) as tc:
        with (
            tc.tile_pool(name="consts", bufs=1) as cpool,
            tc.tile_pool(name="attn_sb", bufs=4) as attn_sb,
            tc.tile_pool(name="attn_ps", bufs=ATTN_PS, space="PSUM") as attn_ps,
            tc.tile_pool(name="ablk", bufs=1) as apool,
            tc.tile_pool(name="cblk", bufs=1) as cpool2,
            tc.tile_pool(name="stage", bufs=2) as stpool,
            tc.tile_pool(name="pp", bufs=PP_BUFS, space="PSUM") as ppool,
        ):
            # ---- constants ----
            wpairs = cpool.tile([128, 3 * COUT], BF16, tag="wpairs", name="wpairs_sb")
            nc.sync.dma_start(out=wpairs[:], in_=wpairs_d[:])
            wb = cpool.tile([128, COUT], BF16, tag="wb", name="wb_sb")
            nc.sync.dma_start(out=wb[:], in_=wb_d[:])
            bias_row = cpool.tile([1, COUT], BF16, tag="bias", name="bias_sb")
            nc.sync.dma_start(out=bias_row[:], in_=bias_d[:])
            node_t = cpool.tile([E, N], F32, tag="node_t", name="node_t_sb")
            nc.sync.dma_start(out=node_t[:], in_=node_t_d[:])
            # wq/wk/wv stored as [E, 2E]: cols 0:E = rows 0:128 of the weight
            # (node half), cols E:2E = rows 128:256 (goal half).
            # wqkv packed [E, 2*3E]: cols [0:3E] = rows 0:128 (node half),
            # cols [3E:6E] = rows 128:256 (goal half); within each, Q|K|V.
            wqkv = cpool.tile([E, 6 * E], F32, tag="wqkv", name="wqkv_sb")
            nc.sync.dma_start(out=wqkv[:, 0:3 * E], in_=wqkv_d[0:E, :])
            nc.sync.dma_start(out=wqkv[:, 3 * E:6 * E], in_=wqkv_d[E:2 * E, :])

            fw_t = cpool.tile([E, COUT], F32, tag="fw_t", name="fw_t_sb")
            nc.sync.dma_start(out=fw_t[:], in_=fw_t_d[:])
            goal = cpool.tile([BPC, E], F32, tag="goal", name="goal_sb")
            nc.sync.dma_start(out=goal[:], in_=goal_d[:])
            ident = cpool.tile([128, 128], F32, tag="ident", name="ident_sb")
            make_identity(nc, ident[:])

            # goal^T [E, BPC] then per-batch broadcast [E, N]
            gt_ps = attn_ps.tile([E, BPC], F32, tag="aps", name="gt_ps")
            nc.tensor.transpose(out=gt_ps[:], in_=goal[:], identity=ident[0:BPC, 0:BPC])
            gt_sb = cpool.tile([E, BPC], F32, tag="gt_sb", name="gt_sb")
            nc.vector.tensor_copy(gt_sb[:], gt_ps[:])
            goal_bc = []
            for b in range(BPC):
                g = cpool.tile([E, N], F32, tag=f"goal_bc{b}", name=f"goal_bc{b}")
                nc.vector.tensor_copy(g[:], gt_sb[:, b:b + 1].to_broadcast([E, N]))
                goal_bc.append(g)

            E3 = 3 * E
            consts = {"node_t": node_t, "goal_bc": goal_bc,
                      "wq_lo": wqkv[:, 0:E], "wq_hi": wqkv[:, E3:E3 + E],
                      "wk_lo": wqkv[:, E:2 * E], "wk_hi": wqkv[:, E3 + E:E3 + 2 * E],
                      "wv_lo": wqkv[:, 2 * E:3 * E], "wv_hi": wqkv[:, E3 + 2 * E:E3 + 3 * E],
                      "fw_t": fw_t, "ident": ident, "bias_row": bias_row}

            # ---- streaming A/C buffers (manual round-robin over NSLOT) ----
            a_tiles = [apool.tile([128, SLEN], BF16, tag=f"a{i}", name=f"ablk{i}")
                       for i in range(NSLOT)]
            # contiguous staging for the state load (full DMA line rate);
            # an engine copy re-strides it into the 130-padded layout.
            st_tiles = [apool.tile([CIN, (SROWS - 1) * W], BF16, tag=f"st{i}",
                                   name=f"stg{i}") for i in range(NSLOT)]
            c_tiles = [cpool2.tile([128, SLEN], BF16, tag=f"c{i}", name=f"cblk{i}")
                       for i in range(NSLOT)]
            b_tiles = [cpool2.tile([128, SLEN], BF16, tag=f"b{i}", name=f"bblk{i}")
                       for i in range(NSLOT)]
            for t in a_tiles:
                nc.gpsimd.memset(t[:], 0.0)
            for t in c_tiles:
                nc.gpsimd.memset(t[96:97, :], 1.0)

            def emit_loads(i):
                b, blk = divmod(i, NBLK)
                r0 = blk * R
                a_t = a_tiles[i % NSLOT]
                c_t = c_tiles[i % NSLOT]
                st_t = st_tiles[i % NSLOT]

                # edge-row zeroing (slots are reused; interior blocks
                # overwrite every row, edge blocks must re-zero the halo)
                if blk == 0:
                    nc.vector.memset(a_t[0:CIN, 1:1 + WP], 0.0)
                    t_lo, row_lo = 1, 0
                    n_rows = SROWS - 1
                elif blk == NBLK - 1:
                    nc.vector.memset(a_t[0:CIN, 1 + (SROWS - 1) * WP:1 + SROWS * WP], 0.0)
                    t_lo, row_lo = 0, r0 - 1
                    n_rows = SROWS - 1
                else:
                    t_lo, row_lo = 0, r0 - 1
                    n_rows = SROWS

                # state rows: contiguous DMA into staging, then an ACT
                # copy re-strides into the padded layout on A[0:64]
                nc.sync.dma_start(out=st_t[:, 0:n_rows * W],
                                  in_=state_d[b][:, row_lo:row_lo + n_rows, :])
                dst = a_t[0:CIN, 1 + t_lo * WP:1 + (t_lo + n_rows) * WP]
                dst = dst.rearrange("p (t x) -> p t x", x=WP)[:, :, 1:1 + W]
                nc.vector.tensor_copy(dst, st_t[:, 0:n_rows * W].rearrange(
                    "p (t x) -> p t x", x=W))
                # duplications: partition-shifted DVE copies (4x/2x modes)
                # keep the DMA engines free for HBM traffic.
                # A upper half: same data shifted one image row (+WP)
                nc.vector.tensor_copy(a_t[64:128, 0:SLEN - WP], a_t[0:64, WP:SLEN])
                # B: [padded state; state shifted one column]
                b_t = b_tiles[i % NSLOT]
                nc.vector.tensor_copy(b_t[0:64, :], a_t[0:64, :])
                nc.vector.tensor_copy(b_t[64:128, 0:SLEN - 1], a_t[0:64, 1:SLEN])
                # C lower half: plain copy of the padded state (on the DMA ring)
                nc.sync.dma_start(out=c_t[0:64, :], in_=a_t[0:64, :])
                # C rows 64..95: one-hot slab, aligned at offset 262
                nc.sync.dma_start(out=c_t[64:64 + N, 262:262 + JBLK],
                                  in_=onehot_d[b][:, r0 * WP:(r0 + R) * WP])

            NBLOCKS = BPC * NBLK
            for i in range(min(NSLOT, NBLOCKS)):
                emit_loads(i)

            # ---- per-batch combo lhsT: [w22; out_node; bias; unused] ----
            combos = []
            for b in range(BPC):
                t = cpool.tile([128, COUT], BF16, tag=f"combo{b}", name=f"combo{b}")
                nc.sync.dma_start(out=t[0:CIN, :], in_=w22_d[:])
                _build_attention(nc, attn_sb, attn_ps, consts, b, t)
                combos.append(t)

            blk_i = 0
            for b in range(BPC):
                for blk in range(NBLK):
                    r0 = blk * R
                    a_t = a_tiles[blk_i % NSLOT]
                    b_t = b_tiles[blk_i % NSLOT]
                    c_t = c_tiles[blk_i % NSLOT]
                    blk_i += 1

                    stage = stpool.tile([128, R * W], BF16 if STORE_BF16 else F32,
                                        tag="stage", name=f"stage{blk_i}")

                    for ti, (j0, wdt) in enumerate(_psum_tiles()):
                        p = ppool.tile([128, NT], F32, tag="pp", name=f"p_{blk_i}_{j0}")
                        # 3x pair taps: [W(0,dx); W(1,dx)] @ A[:, j0+dx]
                        for dx in range(3):
                            nc.tensor.matmul(
                                out=p[:, 0:wdt],
                                lhsT=wpairs[:, dx * COUT:(dx + 1) * COUT],
                                rhs=a_t[:, j0 + dx:j0 + dx + wdt],
                                start=(dx == 0), stop=False,
                            )
                        # dy=2 taps dx=0,1 in one K=128: [W(2,0); W(2,1)] @ B
                        nc.tensor.matmul(
                            out=p[:, 0:wdt],
                            lhsT=wb[:],
                            rhs=b_t[:, j0 + 260:j0 + 260 + wdt],
                            start=False, stop=False,
                        )
                        # K=97: [W(2,2); out_node; bias] @ C[0:97, j0+262]
                        nc.tensor.matmul(
                            out=p[:, 0:wdt],
                            lhsT=combos[b][0:97, :],
                            rhs=c_t[0:97, j0 + 262:j0 + 262 + wdt],
                            start=False, stop=True,
                        )

                        # evacuate PSUM -> stage, dropping pad columns;
                        # alternate ACT/DVE to halve the per-engine load.
                        rows = wdt // WP
                        u0 = j0 // WP
                        pv = p[:, 0:wdt].rearrange("p (u x) -> p u x", x=WP)[:, :, 1:1 + W]
                        stv = stage[:, u0 * W:(u0 + rows) * W].rearrange(
                            "p (u x) -> p u x", x=W)
                        if EVAC_MOD == 0 or ti % EVAC_MOD == 0:
                            nc.scalar.copy(stv, pv)
                        else:
                            nc.vector.tensor_copy(stv, pv)

                    # store two contiguous f32 chunks on the ACT HWDGE ring
                    # (keeps the SP ring free for the next block's loads)
                    hw = R * W // 2
                    nc.scalar.dma_start(out=out_d[b][:, r0 * W:r0 * W + hw],
                                        in_=stage[:, 0:hw])
                    nc.scalar.dma_start(out=out_d[b][:, r0 * W + hw:(r0 + R) * W],
                                        in_=stage[:, hw:R * W])

                    if blk_i - 1 + NSLOT < NBLOCKS:
                        emit_loads(blk_i - 1 + NSLOT)

    nc.finalize()
    return nc


def _prepare_inputs(inputs):
    gb = np.asarray(inputs["game_board"]).astype(np.int64)
    state = np.asarray(inputs["state"], dtype=np.float32)
    node_embeds = np.asarray(inputs["node_embeds"], dtype=np.float32)
    goal_embed = np.asarray(inputs["goal_embed"], dtype=np.float32)
    char_to_node = np.asarray(inputs["char_to_node"]).astype(np.int64)
    conv_w = np.asarray(inputs["conv_w"], dtype=np.float32)
    conv_b = np.asarray(inputs["conv_b"], dtype=np.float32)
    wqkv = np.ascontiguousarray(np.concatenate([
        np.asarray(inputs["wQ"], dtype=np.float32),
        np.asarray(inputs["wK"], dtype=np.float32),
        np.asarray(inputs["wV"], dtype=np.float32)], axis=1))
    final_w = np.asarray(inputs["final_w"], dtype=np.float32)
    final_b = np.asarray(inputs["final_b"], dtype=np.float32)

    # host-side index preprocessing: node index per cell (+ validity mask)
    valid = (gb >= 0) & (gb < N)
    idx = char_to_node[np.clip(gb, 0, N - 1)]
    idx = np.clip(idx, 0, N - 1)

    # one-hot [B, N, H, WP] bf16 (pad columns stay zero)
    onehot = np.zeros((B, N, H, WP), dtype=ml_dtypes.bfloat16)
    bb = np.arange(B)[:, None, None]
    yy = np.arange(H)[None, :, None]
    xx = np.arange(W)[None, None, :]
    onehot[bb, idx, yy, xx + 1] = valid.astype(ml_dtypes.bfloat16)
    onehot = onehot.reshape(B, N, H * WP)

    # conv tap lhsT layouts (bf16)
    wt = conv_w.transpose(1, 2, 3, 0)  # [CIN, 3, 3, COUT]
    wpairs = np.concatenate([
        np.concatenate([wt[:, 0, dx, :], wt[:, 1, dx, :]], axis=0)
        for dx in range(3)], axis=1).astype(ml_dtypes.bfloat16)   # [128, 3*COUT]
    wb = np.concatenate([wt[:, 2, 0, :], wt[:, 2, 1, :]],
                        axis=0).astype(ml_dtypes.bfloat16)        # [128, COUT]
    w22 = np.ascontiguousarray(wt[:, 2, 2, :]).astype(ml_dtypes.bfloat16)  # [64, COUT]

    bias_row = (conv_b + final_b).reshape(1, COUT).astype(ml_dtypes.bfloat16)
    node_t = np.ascontiguousarray(node_embeds.T)
    fw_t = np.ascontiguousarray(final_w.T)
    state_bf = state.astype(ml_dtypes.bfloat16)

    in_maps = []
    for c in range(NCORES):
        sl = slice(c * BPC, (c + 1) * BPC)
        in_maps.append({
            "state": np.ascontiguousarray(state_bf[sl]),
            "onehot": np.ascontiguousarray(onehot[sl]),
            "wpairs": wpairs,
            "wb": np.ascontiguousarray(wb),
            "w22": w22,
            "bias_row": bias_row,
            "goal": np.ascontiguousarray(goal_embed[sl]),
            "node_t": node_t,
            "wqkv": wqkv,
            "fw_t": fw_t,
        })
    return in_maps


def kernel(**inputs):
    global LAST_RESULTS
    if "nc" not in _CACHE:
        _CACHE["nc"] = _build_kernel()
    nc = _CACHE["nc"]
    in_maps = _prepare_inputs(inputs)
    res = run_bass_kernel_spmd(
        nc, in_maps, list(range(NCORES)),
        trace=bool(os.environ.get("BASS_TRACE")),
    )
    LAST_RESULTS = res
    out = np.concatenate([r["out"].reshape(BPC, COUT, H, W) for r in res.results], axis=0)
    if out.dtype != np.float32:
        out = out.astype(np.float32)
    return np.ascontiguousarray(out, dtype=np.float32)

